# revision 15
# baseline (speedup 1.0000x reference)
"""GAT layer (N=8192, D=64) as a Bass/Tile kernel on 8 TRN2 NeuronCores.

Math (reference):
    h  = x @ W.T + b
    s1 = h @ a1 ; s2 = h @ a2                    # [N] each
    score[i,j] = s2[i] + s1[j]
    att = softmax_j(leaky_relu(score))
    out = att @ x

Reformulation used here:
    Fold the linear layer:  v = W.T @ [a1|a2], c_k = b.a_k
      p1 = x @ v1 ; p2 = x @ v2 ; s1 = p1 + c1 ; s2 = p2 + c2
    Softmax rows are shift invariant, so subtract p2[i] from row i:
      exp(lr(score) - p2[i]) = max( exp(sh1[j]),
                                    exp(0.01*sh1[j]) * exp(-0.99*p2[i]) )
      with sh1[j] = p1[j] + c1 + c2   (lr = leaky-relu, exp is monotone
      so exp(max(a,b)) = max(exp a, exp b))
    So with per-j-row scalars E1 = exp(sh1), F1 = exp(0.01*sh1) and a
    broadcast tile G2b[j,i] = exp(-0.99*p2[i]), the unnormalized weight
    tile (layout [j partitions, i free]) is ONE tensor_scalar op:
      e[j,i] = max( G2b[j,i] * F1[j],  E1[j] )
    The final matmul (with a ones-column appended to x to get the
    softmax denominator for free) accumulates over j in PSUM:
      outT[0:64, i] += x_ext[j,:].T @ e[j, i] ; Z[i] = outT[64, i]

Sharding: each core owns N/8 = 1024 query rows i (full x is only 2MB and
is replicated to every core), no collectives needed.
"""

import sys
import types

import ml_dtypes
import numpy as np

import concourse.bass as bass
import concourse.bacc as bacc
import concourse.mybir as mybir
import concourse.tile as tile
from concourse.masks import make_identity
from concourse.bass_utils import run_bass_kernel_spmd


def _install_ntff_hook_shim():
    """The agent image's ``antenv`` lacks ``axon_hooks``; provide it so
    ``run_bass_kernel_spmd(trace=True)`` can capture NTFF profiles. The
    actual hook implementation ships with the axon boot package."""
    if "antenv.axon_hooks" in sys.modules:
        return
    try:
        from trn_agent_boot.trn_boot import _ntff_profile_via_ctypes

        hook = _ntff_profile_via_ctypes("/opt/axon/libaxon_pjrt.so")
        mod = types.ModuleType("antenv.axon_hooks")
        mod._hook = hook
        mod.get_axon_ntff_profile_hook = lambda: mod._hook
        mod.set_axon_ntff_profile_hook = lambda h: setattr(mod, "_hook", h)
        sys.modules["antenv.axon_hooks"] = mod
    except Exception:
        pass


_install_ntff_hook_shim()

N, D = 8192, 64
NCORES = 8
RB = N // NCORES          # rows (i) per core = 1024
NT = N // 128             # j tiles of 128 = 64
BT = RB // 128            # i tiles per core = 8
F32 = mybir.dt.float32
BF16 = mybir.dt.bfloat16
EXP = mybir.ActivationFunctionType.Exp
ADD = mybir.AluOpType.add
MUL = mybir.AluOpType.mult
MAX = mybir.AluOpType.max
AX_X = mybir.AxisListType.X


def build_bass() -> bass.Bass:
    nc = bacc.Bacc(None)
    x_d = nc.declare_dram_parameter("x", [N, D], F32, isOutput=False)
    xbf_d = nc.declare_dram_parameter("xbf", [N, D + 1], BF16, isOutput=False)
    xb_d = nc.declare_dram_parameter("xblk", [RB, D], F32, isOutput=False)
    W_d = nc.declare_dram_parameter("W", [D, D], F32, isOutput=False)
    b_d = nc.declare_dram_parameter("b", [D, 1], F32, isOutput=False)
    a_d = nc.declare_dram_parameter("a", [2 * D, 1], F32, isOutput=False)
    out_d = nc.declare_dram_parameter("out", [RB, D], F32, isOutput=True)

    with tile.TileContext(nc) as tc:
        with (
            tc.tile_pool(name="persist", bufs=1) as persist,
            tc.tile_pool(name="small", bufs=1) as small,
            tc.tile_pool(name="work", bufs=2) as work,
            tc.tile_pool(name="epool", bufs=3) as epool,
            tc.tile_pool(name="opool", bufs=2) as opool,
            tc.tile_pool(name="psumA", bufs=3, space="PSUM") as psumA,
            tc.tile_pool(name="psumB", bufs=1, space="PSUM") as psumB,
        ):
            # ---------------- small constants ----------------
            W_sb = small.tile([D, D], F32)
            nc.sync.dma_start(W_sb, W_d[:, :])
            b_sb = small.tile([D, 1], F32)
            nc.sync.dma_start(b_sb, b_d[:, :])
            a_sb = small.tile([D, 2], F32)
            nc.sync.dma_start(
                a_sb,
                bass.AP(
                    tensor=a_d[:, :].tensor,
                    offset=a_d[:, :].offset,
                    ap=[[1, D], [D, 2]],
                ),
            )
            ones_row = small.tile([1, 128], F32)
            nc.vector.memset(ones_row, 1.0)
            ident = small.tile([128, 128], F32)
            make_identity(nc, ident)
            # bounce the small operands through one engine (DVE) so PE
            # matmuls depend on a single semaphore (HW LDW wait-slot limit)
            W2 = small.tile([D, D], F32)
            nc.vector.tensor_copy(out=W2, in_=W_sb)
            a2 = small.tile([D, 2], F32)
            nc.vector.tensor_copy(out=a2, in_=a_sb)
            b2 = small.tile([D, 1], F32)
            nc.vector.tensor_copy(out=b2, in_=b_sb)

            # ---------------- x loads ----------------
            # x_sb[p, t, d] = x[t*128 + p, d]  (f32, for the s1 projection)
            x_sb = persist.tile([128, NT, D], F32)
            x_src = x_d[:, :].rearrange("(t p) d -> p t d", p=128)
            for c in range(8):
                nc.sync.dma_start(
                    x_sb[:, 8 * c : 8 * (c + 1), :],
                    x_src[:, 8 * c : 8 * (c + 1), :],
                )
            # bf16 x with ones column (host-prepared) for the PE matmul
            x_bf = persist.tile([128, NT, D + 1], BF16)
            xbf_src = xbf_d[:, :].rearrange("(t p) d -> p t d", p=128)
            for c in range(8):
                nc.sync.dma_start(
                    x_bf[:, 8 * c : 8 * (c + 1), :],
                    xbf_src[:, 8 * c : 8 * (c + 1), :],
                )

            # this core's 1024 query rows
            xblk_sb = small.tile([128, BT, D], F32)
            nc.sync.dma_start(
                xblk_sb, xb_d[:, :].rearrange("(t p) d -> p t d", p=128)
            )

            # ---------------- tiny projections on PE ----------------
            # v = W.T @ [a1|a2]  [64,2] ;  c = [b.a1, b.a2]  [1,2]
            v_ps = psumA.tile([D, 2], F32, tag="ps", name="v_ps")
            nc.tensor.matmul(v_ps, lhsT=W2, rhs=a2, start=True, stop=True)
            v_sb = small.tile([D, 2], F32)
            nc.vector.tensor_copy(out=v_sb, in_=v_ps)

            c_ps = psumA.tile([1, 2], F32, tag="ps", name="c_ps")
            nc.tensor.matmul(c_ps, lhsT=b2, rhs=a2, start=True, stop=True)
            c_sb = small.tile([1, 2], F32)
            nc.vector.tensor_copy(out=c_sb, in_=c_ps)

            # c12 = (c1 + c2) broadcast down 128 partitions
            cb_ps = psumA.tile([128, 2], F32, tag="ps", name="cb_ps")
            nc.tensor.matmul(cb_ps, lhsT=ones_row, rhs=c_sb, start=True, stop=True)
            c12 = small.tile([128, 1], F32)
            nc.vector.tensor_reduce(out=c12, in_=cb_ps, axis=AX_X, op=ADD)
            c12s = small.tile([128, 1], F32)
            nc.vector.tensor_scalar(
                out=c12s, in0=c12, scalar1=0.01, scalar2=None, op0=MUL
            )

            # v1 broadcast down partitions: v1b[p, d] = v[d, 0]
            vT_ps = psumA.tile([2, D], F32, tag="ps", name="vT_ps")
            nc.tensor.transpose(vT_ps, v_sb, ident[:D, :D])
            vT_sb = small.tile([2, D], F32)
            nc.vector.tensor_copy(out=vT_sb, in_=vT_ps)
            v1b_ps = psumA.tile([128, D], F32, tag="ps", name="v1b_ps")
            nc.tensor.matmul(
                v1b_ps, lhsT=ones_row, rhs=vT_sb[0:1, :], start=True, stop=True
            )
            v1b = small.tile([128, D], F32)
            nc.vector.tensor_copy(out=v1b, in_=v1b_ps)

            # ---------------- p2 for this block -> G2b ----------------
            # x_blk.T via 8 PE transposes -> [64, 1024]
            xblkT = small.tile([D, RB], F32)
            for t in range(BT):
                tp = psumA.tile([D, 128], F32, tag="ps", name="tp")
                nc.tensor.transpose(tp, xblk_sb[:, t, :], ident)
                nc.scalar.copy(out=xblkT[:, t * 128 : (t + 1) * 128], in_=tp)

            # p2row (raw x_blk @ v2) then g2row = exp(-0.99 * p2row)
            g2row = small.tile([1, RB], F32)
            for h in range(2):
                p2_ps = psumA.tile([1, 512], F32, tag="ps", name="p2_ps")
                nc.tensor.matmul(
                    p2_ps,
                    lhsT=v_sb[:, 1:2],
                    rhs=xblkT[:, h * 512 : (h + 1) * 512],
                    start=True,
                    stop=True,
                )
                nc.scalar.activation(
                    out=g2row[:, h * 512 : (h + 1) * 512],
                    in_=p2_ps,
                    func=EXP,
                    scale=-0.99,
                )
            # broadcast to 128 partitions, cast bf16
            G2b = persist.tile([128, RB], BF16)
            for h in range(2):
                gb_ps = psumA.tile([128, 512], F32, tag="ps", name="gb_ps")
                nc.tensor.matmul(
                    gb_ps,
                    lhsT=ones_row,
                    rhs=g2row[:, h * 512 : (h + 1) * 512],
                    start=True,
                    stop=True,
                )
                nc.vector.tensor_copy(
                    out=G2b[:, h * 512 : (h + 1) * 512], in_=gb_ps
                )

            # ---------------- s1 columns + exps ----------------
            # s1c[p, jt] = sum_d x[jt*128+p, d] * v1[d]
            s1c = small.tile([128, NT], F32)
            E1c = small.tile([128, NT], F32)
            F1c = small.tile([128, NT], F32)
            v1b_b = bass.AP(
                tensor=v1b.tensor,
                offset=v1b.offset,
                ap=[v1b.ap[0], [0, 8], v1b.ap[1]],
            )
            for c in range(8):
                tmp = work.tile([128, 8, D], F32, tag="tmp", name="tmp")
                nc.vector.tensor_mul(
                    tmp, x_sb[:, 8 * c : 8 * (c + 1), 0:D], v1b_b
                )
                nc.vector.tensor_reduce(
                    out=s1c[:, 8 * c : 8 * (c + 1)], in_=tmp, axis=AX_X, op=ADD
                )
            for c in range(2):
                nc.scalar.activation(
                    out=E1c[:, 32 * c : 32 * (c + 1)],
                    in_=s1c[:, 32 * c : 32 * (c + 1)],
                    func=EXP,
                    bias=c12,
                    scale=1.0,
                )
                nc.scalar.activation(
                    out=F1c[:, 32 * c : 32 * (c + 1)],
                    in_=s1c[:, 32 * c : 32 * (c + 1)],
                    func=EXP,
                    bias=c12s,
                    scale=0.01,
                )

            # ---------------- main loop over j tiles ----------------
            acc0 = psumB.tile([D + 1, 512], F32, tag="acc0", name="acc0")
            acc1 = psumB.tile([D + 1, 512], F32, tag="acc1", name="acc1")
            accs = [acc0, acc1]
            for jt in range(NT):
                e_t = epool.tile([128, RB], BF16, tag="e", name="e_t")
                # e[j,i] = max(G2b[j,i] * F1[j], E1[j])
                nc.vector.tensor_scalar(
                    out=e_t,
                    in0=G2b,
                    scalar1=F1c[:, jt : jt + 1],
                    scalar2=E1c[:, jt : jt + 1],
                    op0=MUL,
                    op1=MAX,
                )
                for h in range(2):
                    nc.tensor.matmul(
                        accs[h],
                        lhsT=x_bf[:, jt, :],
                        rhs=e_t[:, h * 512 : (h + 1) * 512],
                        start=(jt == 0),
                        stop=(jt == NT - 1),
                    )

            # ---------------- epilogue: normalize + store ----------------
            outT = small.tile([D + 1, RB], F32)
            for h in range(2):
                nc.vector.tensor_copy(
                    out=outT[:, h * 512 : (h + 1) * 512], in_=accs[h]
                )
            for t in range(BT):
                tp2 = psumA.tile([128, D + 1], F32, tag="ps", name="tp2")
                nc.tensor.transpose(
                    tp2, outT[:, t * 128 : (t + 1) * 128], ident[: D + 1, : D + 1]
                )
                rcol = opool.tile([128, 1], F32, tag="rcol", name="rcol")
                nc.vector.reciprocal(rcol, tp2[:, D : D + 1])
                o_t = opool.tile([128, D], F32, tag="ot", name="o_t")
                nc.vector.tensor_scalar(
                    out=o_t, in0=tp2[:, 0:D], scalar1=rcol, scalar2=None, op0=MUL
                )
                nc.sync.dma_start(out_d[t * 128 : (t + 1) * 128, :], o_t)

    nc.finalize()
    return nc


def _execute(inputs: dict, trace: bool = False):
    x = np.ascontiguousarray(np.asarray(inputs["x"], dtype=np.float32))
    W = np.ascontiguousarray(np.asarray(inputs["W"], dtype=np.float32))
    b = np.ascontiguousarray(
        np.asarray(inputs["b"], dtype=np.float32).reshape(D, 1)
    )
    a = np.ascontiguousarray(
        np.asarray(inputs["a"], dtype=np.float32).reshape(2 * D, 1)
    )
    assert x.shape == (N, D) and W.shape == (D, D)

    xbf = np.ascontiguousarray(
        np.concatenate([x, np.ones((N, 1), np.float32)], axis=1).astype(
            ml_dtypes.bfloat16
        )
    )

    nc = build_bass()
    in_maps = []
    for c in range(NCORES):
        in_maps.append(
            {
                "x": x,
                "xbf": xbf,
                "xblk": np.ascontiguousarray(x[c * RB : (c + 1) * RB]),
                "W": W,
                "b": b,
                "a": a,
            }
        )
    res = run_bass_kernel_spmd(
        nc, in_maps, core_ids=list(range(NCORES)), trace=trace
    )
    out = np.concatenate([r["out"] for r in res.results], axis=0)
    return out, res


def kernel(x, W, b, a):
    out, _ = _execute({"x": x, "W": W, "b": b, "a": a})
    return out


# revision 16
# speedup vs baseline: 1.1785x; 1.1785x over previous
"""GAT layer (N=8192, D=64) as a Bass/Tile kernel on 8 TRN2 NeuronCores.

Math (reference):
    h  = x @ W.T + b
    s1 = h @ a1 ; s2 = h @ a2                    # [N] each
    score[i,j] = s2[i] + s1[j]
    att = softmax_j(leaky_relu(score))
    out = att @ x

Reformulation used here:
    Fold the linear layer:  v = W.T @ [a1|a2], c_k = b.a_k
      p1 = x @ v1 ; p2 = x @ v2 ; s1 = p1 + c1 ; s2 = p2 + c2
    Softmax rows are shift invariant, so subtract p2[i] from row i:
      exp(lr(score) - p2[i]) = max( exp(sh1[j]),
                                    exp(0.01*sh1[j]) * exp(-0.99*p2[i]) )
      with sh1[j] = p1[j] + c1 + c2   (lr = leaky-relu, exp is monotone
      so exp(max(a,b)) = max(exp a, exp b))
    So with per-j-row scalars E1 = exp(sh1), F1 = exp(0.01*sh1) and a
    broadcast tile G2b[j,i] = exp(-0.99*p2[i]), the unnormalized weight
    tile (layout [j partitions, i free]) is ONE tensor_scalar op:
      e[j,i] = max( G2b[j,i] * F1[j],  E1[j] )
    The final matmul (with a ones-column appended to x to get the
    softmax denominator for free) accumulates over j in PSUM:
      outT[0:64, i] += x_ext[j,:].T @ e[j, i] ; Z[i] = outT[64, i]

Sharding: each core owns N/8 = 1024 query rows i (full x is only 2MB and
is replicated to every core), no collectives needed. Inputs are shipped
pre-permuted to partition-major layout (p, t, d) so every DMA is
contiguous on both sides, and are spread over several engine DMA queues.
"""

import sys
import types

import ml_dtypes
import numpy as np

import concourse.bacc as bacc
import concourse.bass as bass
import concourse.mybir as mybir
import concourse.tile as tile
from concourse.masks import make_identity
from concourse.bass_utils import run_bass_kernel_spmd


def _install_ntff_hook_shim():
    """The agent image's ``antenv`` lacks ``axon_hooks``; provide it so
    ``run_bass_kernel_spmd(trace=True)`` can capture NTFF profiles. The
    actual hook implementation ships with the axon boot package."""
    if "antenv.axon_hooks" in sys.modules:
        return
    try:
        from trn_agent_boot.trn_boot import _ntff_profile_via_ctypes

        hook = _ntff_profile_via_ctypes("/opt/axon/libaxon_pjrt.so")
        mod = types.ModuleType("antenv.axon_hooks")
        mod._hook = hook
        mod.get_axon_ntff_profile_hook = lambda: mod._hook
        mod.set_axon_ntff_profile_hook = lambda h: setattr(mod, "_hook", h)
        sys.modules["antenv.axon_hooks"] = mod
    except Exception:
        pass


_install_ntff_hook_shim()

N, D = 8192, 64
NCORES = 8
RB = N // NCORES          # rows (i) per core = 1024
NT = N // 128             # j tiles of 128 = 64
BT = RB // 128            # i tiles per core = 8
F32 = mybir.dt.float32
BF16 = mybir.dt.bfloat16
EXP = mybir.ActivationFunctionType.Exp
ADD = mybir.AluOpType.add
MUL = mybir.AluOpType.mult
MAX = mybir.AluOpType.max
AX_X = mybir.AxisListType.X


def build_bass() -> bass.Bass:
    nc = bacc.Bacc(None)
    # partition-major (p, t, d) layouts, prepared on the host
    xp_d = nc.declare_dram_parameter("xp", [128, NT * D], F32, isOutput=False)
    xbf_d = nc.declare_dram_parameter(
        "xbf", [128, NT * (D + 1)], BF16, isOutput=False
    )
    xbk_d = nc.declare_dram_parameter("xblk", [128, BT * D], F32, isOutput=False)
    W_d = nc.declare_dram_parameter("W", [D, D], F32, isOutput=False)
    b_d = nc.declare_dram_parameter("b", [D, 1], F32, isOutput=False)
    a_d = nc.declare_dram_parameter("a", [2 * D, 1], F32, isOutput=False)
    out_d = nc.declare_dram_parameter("out", [128, BT * D], F32, isOutput=True)

    with tile.TileContext(nc) as tc:
        with (
            tc.tile_pool(name="persist", bufs=1) as persist,
            tc.tile_pool(name="small", bufs=1) as small,
            tc.tile_pool(name="work", bufs=2) as work,
            tc.tile_pool(name="epool", bufs=3) as epool,
            tc.tile_pool(name="opool", bufs=2) as opool,
            tc.tile_pool(name="psumA", bufs=3, space="PSUM") as psumA,
            tc.tile_pool(name="psumB", bufs=1, space="PSUM") as psumB,
        ):
            # ------- small loads + this core's rows (gpsimd SW queue) -------
            xblk_sb = small.tile([128, BT, D], F32)
            nc.gpsimd.dma_start(
                xblk_sb, xbk_d[:, :].rearrange("p (t d) -> p t d", t=BT)
            )
            W_sb = small.tile([D, D], F32)
            nc.gpsimd.dma_start(W_sb, W_d[:, :])
            b_sb = small.tile([D, 1], F32)
            nc.gpsimd.dma_start(b_sb, b_d[:, :])
            a_sb = small.tile([D, 2], F32)
            nc.gpsimd.dma_start(
                a_sb,
                bass.AP(
                    tensor=a_d[:, :].tensor,
                    offset=a_d[:, :].offset,
                    ap=[[1, D], [D, 2]],
                ),
            )
            ones_row = small.tile([1, 128], F32)
            nc.vector.memset(ones_row, 1.0)
            ident = small.tile([128, 128], F32)
            make_identity(nc, ident)

            # ------- x loads: f32 via sync queue, bf16 via scalar queue -----
            x_sb = persist.tile([128, NT, D], F32)
            x_src = xp_d[:, :].rearrange("p (t d) -> p t d", t=NT)
            for c in range(8):
                nc.sync.dma_start(
                    x_sb[:, 8 * c : 8 * (c + 1), :],
                    x_src[:, 8 * c : 8 * (c + 1), :],
                )
            x_bf = persist.tile([128, NT, D + 1], BF16)
            xbf_src = xbf_d[:, :].rearrange("p (t d) -> p t d", t=NT)
            for c in range(8):
                nc.scalar.dma_start(
                    x_bf[:, 8 * c : 8 * (c + 1), :],
                    xbf_src[:, 8 * c : 8 * (c + 1), :],
                )

            # ---------------- tiny projections on PE ----------------
            # v = W.T @ [a1|a2]  [64,2] ;  c = [b.a1, b.a2]  [1,2]
            v_ps = psumA.tile([D, 2], F32, tag="ps", name="v_ps")
            nc.tensor.matmul(v_ps, lhsT=W_sb, rhs=a_sb, start=True, stop=True)
            v_sb = small.tile([D, 2], F32)
            nc.vector.tensor_copy(out=v_sb, in_=v_ps)

            c_ps = psumA.tile([1, 2], F32, tag="ps", name="c_ps")
            nc.tensor.matmul(c_ps, lhsT=b_sb, rhs=a_sb, start=True, stop=True)
            c_sb = small.tile([1, 2], F32)
            nc.vector.tensor_copy(out=c_sb, in_=c_ps)

            # c12 = (c1 + c2) broadcast down 128 partitions
            cb_ps = psumA.tile([128, 2], F32, tag="ps", name="cb_ps")
            nc.tensor.matmul(cb_ps, lhsT=ones_row, rhs=c_sb, start=True, stop=True)
            c12 = small.tile([128, 1], F32)
            nc.vector.tensor_reduce(out=c12, in_=cb_ps, axis=AX_X, op=ADD)
            c12s = small.tile([128, 1], F32)
            nc.vector.tensor_scalar(
                out=c12s, in0=c12, scalar1=0.01, scalar2=None, op0=MUL
            )

            # v1 broadcast down partitions: v1b[p, d] = v[d, 0]
            vT_ps = psumA.tile([2, D], F32, tag="ps", name="vT_ps")
            nc.tensor.transpose(vT_ps, v_sb, ident[:D, :D])
            vT_sb = small.tile([2, D], F32)
            nc.vector.tensor_copy(out=vT_sb, in_=vT_ps)
            v1b_ps = psumA.tile([128, D], F32, tag="ps", name="v1b_ps")
            nc.tensor.matmul(
                v1b_ps, lhsT=ones_row, rhs=vT_sb[0:1, :], start=True, stop=True
            )
            v1b = small.tile([128, D], F32)
            nc.vector.tensor_copy(out=v1b, in_=v1b_ps)

            # ---------------- p2 for this block -> G2b ----------------
            # x_blk.T via 8 PE transposes -> [64, 1024]
            xblkT = small.tile([D, RB], F32)
            for t in range(BT):
                tp = psumA.tile([D, 128], F32, tag="ps", name="tp")
                nc.tensor.transpose(tp, xblk_sb[:, t, :], ident)
                nc.scalar.copy(out=xblkT[:, t * 128 : (t + 1) * 128], in_=tp)

            # p2row (raw x_blk @ v2) then g2row = exp(-0.99 * p2row)
            g2row = small.tile([1, RB], F32)
            for h in range(2):
                p2_ps = psumA.tile([1, 512], F32, tag="ps", name="p2_ps")
                nc.tensor.matmul(
                    p2_ps,
                    lhsT=v_sb[:, 1:2],
                    rhs=xblkT[:, h * 512 : (h + 1) * 512],
                    start=True,
                    stop=True,
                )
                nc.scalar.activation(
                    out=g2row[:, h * 512 : (h + 1) * 512],
                    in_=p2_ps,
                    func=EXP,
                    scale=-0.99,
                )
            # broadcast to 128 partitions, cast bf16
            G2b = persist.tile([128, RB], BF16)
            for h in range(2):
                gb_ps = psumA.tile([128, 512], F32, tag="ps", name="gb_ps")
                nc.tensor.matmul(
                    gb_ps,
                    lhsT=ones_row,
                    rhs=g2row[:, h * 512 : (h + 1) * 512],
                    start=True,
                    stop=True,
                )
                nc.vector.tensor_copy(
                    out=G2b[:, h * 512 : (h + 1) * 512], in_=gb_ps
                )

            # ---------------- s1 columns + exps ----------------
            # s1c[p, jt] = sum_d x[jt*128+p, d] * v1[d]
            s1c = small.tile([128, NT], F32)
            E1c = small.tile([128, NT], F32)
            F1c = small.tile([128, NT], F32)
            v1b_b = bass.AP(
                tensor=v1b.tensor,
                offset=v1b.offset,
                ap=[v1b.ap[0], [0, 8], v1b.ap[1]],
            )
            for c in range(8):
                tmp = work.tile([128, 8, D], F32, tag="tmp", name="tmp")
                nc.vector.tensor_mul(
                    tmp, x_sb[:, 8 * c : 8 * (c + 1), :], v1b_b
                )
                nc.vector.tensor_reduce(
                    out=s1c[:, 8 * c : 8 * (c + 1)], in_=tmp, axis=AX_X, op=ADD
                )
            for c in range(4):
                nc.scalar.activation(
                    out=E1c[:, 16 * c : 16 * (c + 1)],
                    in_=s1c[:, 16 * c : 16 * (c + 1)],
                    func=EXP,
                    bias=c12,
                    scale=1.0,
                )
                nc.scalar.activation(
                    out=F1c[:, 16 * c : 16 * (c + 1)],
                    in_=s1c[:, 16 * c : 16 * (c + 1)],
                    func=EXP,
                    bias=c12s,
                    scale=0.01,
                )

            # ---------------- main loop over j tiles ----------------
            acc0 = psumB.tile([D + 1, 512], F32, tag="acc0", name="acc0")
            acc1 = psumB.tile([D + 1, 512], F32, tag="acc1", name="acc1")
            accs = [acc0, acc1]
            for jt in range(NT):
                e_t = epool.tile([128, RB], BF16, tag="e", name="e_t")
                # e[j,i] = max(G2b[j,i] * F1[j], E1[j])
                nc.vector.tensor_scalar(
                    out=e_t,
                    in0=G2b,
                    scalar1=F1c[:, jt : jt + 1],
                    scalar2=E1c[:, jt : jt + 1],
                    op0=MUL,
                    op1=MAX,
                )
                for h in range(2):
                    nc.tensor.matmul(
                        accs[h],
                        lhsT=x_bf[:, jt, :],
                        rhs=e_t[:, h * 512 : (h + 1) * 512],
                        start=(jt == 0),
                        stop=(jt == NT - 1),
                    )

            # ---------------- epilogue: normalize + store ----------------
            outT = small.tile([D + 1, RB], F32)
            for h in range(2):
                nc.vector.tensor_copy(
                    out=outT[:, h * 512 : (h + 1) * 512], in_=accs[h]
                )
            out_sb = small.tile([128, BT, D], F32)
            for t in range(BT):
                tp2 = psumA.tile([128, D + 1], F32, tag="ps", name="tp2")
                nc.tensor.transpose(
                    tp2, outT[:, t * 128 : (t + 1) * 128], ident[: D + 1, : D + 1]
                )
                rcol = opool.tile([128, 1], F32, tag="rcol", name="rcol")
                nc.vector.reciprocal(rcol, tp2[:, D : D + 1])
                nc.vector.tensor_scalar(
                    out=out_sb[:, t, :],
                    in0=tp2[:, 0:D],
                    scalar1=rcol,
                    scalar2=None,
                    op0=MUL,
                )
            nc.sync.dma_start(
                out_d[:, :].rearrange("p (t d) -> p t d", t=BT), out_sb
            )

    nc.finalize()
    return nc


def _execute(inputs: dict, trace: bool = False):
    x = np.ascontiguousarray(np.asarray(inputs["x"], dtype=np.float32))
    W = np.ascontiguousarray(np.asarray(inputs["W"], dtype=np.float32))
    b = np.ascontiguousarray(
        np.asarray(inputs["b"], dtype=np.float32).reshape(D, 1)
    )
    a = np.ascontiguousarray(
        np.asarray(inputs["a"], dtype=np.float32).reshape(2 * D, 1)
    )
    assert x.shape == (N, D) and W.shape == (D, D)

    # partition-major permutations: (t*128+p, d) -> (p, t*D+d)
    xp = np.ascontiguousarray(
        x.reshape(NT, 128, D).transpose(1, 0, 2).reshape(128, NT * D)
    )
    xe = np.concatenate([x, np.ones((N, 1), np.float32)], axis=1)
    xbf = np.ascontiguousarray(
        xe.reshape(NT, 128, D + 1)
        .transpose(1, 0, 2)
        .reshape(128, NT * (D + 1))
        .astype(ml_dtypes.bfloat16)
    )

    nc = build_bass()
    in_maps = []
    for c in range(NCORES):
        xblk = x[c * RB : (c + 1) * RB]
        xbk = np.ascontiguousarray(
            xblk.reshape(BT, 128, D).transpose(1, 0, 2).reshape(128, BT * D)
        )
        in_maps.append(
            {"xp": xp, "xbf": xbf, "xblk": xbk, "W": W, "b": b, "a": a}
        )
    res = run_bass_kernel_spmd(
        nc, in_maps, core_ids=list(range(NCORES)), trace=trace
    )
    # un-permute each core's output: (p, t*D+d) -> (t*128+p, d)
    outs = []
    for r in res.results:
        o = r["out"].reshape(128, BT, D).transpose(1, 0, 2).reshape(RB, D)
        outs.append(o)
    out = np.ascontiguousarray(np.concatenate(outs, axis=0))
    return out, res


def kernel(x, W, b, a):
    out, _ = _execute({"x": x, "W": W, "b": b, "a": a})
    return out
